# revision 1
# baseline (speedup 1.0000x reference)
"""Trainium2 Bass kernel for JacobianMLP.

Computes, for x:[B,16], per-head weights W1:[16,512,16], b1:[16,512],
W2:[16,512], b2:[16]:
    h   = einsum('bi,ohi->boh', x, W1) + b1
    h   = leaky_relu(h, 0.2)
    out = einsum('boh,oh->bo', h, W2) + b2

Strategy (8 NeuronCores, data-parallel over batch):
  leaky(h) = 0.2*h + 0.8*relu(h), so
  out = [0.2*W2^T(W1 x + b1) + b2]  (tiny 17x16 folded matmul on x)
      + (0.8*W2)^T relu(W1 x + b1)  (main path)

Per core (batch 4096, tiles of 512):
  L1: 4-way row-tiled f32r matmuls, W1 stationary [K=32(17 used), M=128],
      x^T streams (with a ones-row so b1 rides the matmul) -> PSUM
      [128 hid, 512 batch] chunks.
  relu: ACT(Relu) / DVE(tensor_scalar_max) PSUM->SBUF, split across both.
  L2: 4-way col-tiled f32r matmuls (0.8*W2 blocks, M=32 zero-padded)
      accumulating 4 partition-slices; a constant 0/1 collapse matmul +
      the tiny folded matmul sum everything into one [16,512] accumulator.
  Output [16, 4096] per core; host transposes/concats.
"""

import sys

for _p in ("/opt/trn_rl_repo",):
    if _p not in sys.path:
        sys.path.insert(0, _p)

import numpy as np

B, I, O, H = 32768, 16, 16, 512
NCORES = 8
BC = B // NCORES          # batch per core = 4096
TB = 512                  # batch tile (matmul moving dim)
NT = BC // TB             # batch tiles per core = 8
NH = O * H                # flat hidden = 8192
NCHUNK = NH // 128        # 64 hid chunks of 128
NROUND = NCHUNK // 4      # 16 rounds, 4 row-tiles each

_cache = {}


def _build(reps=1):
    key = ("nc", reps)
    if key in _cache:
        return _cache[key]

    import concourse.bacc as bacc
    import concourse.tile as tile
    from concourse import mybir

    f32 = mybir.dt.float32
    f32r = mybir.dt.float32r
    bf16 = mybir.dt.bfloat16
    Relu = mybir.ActivationFunctionType.Relu

    nc = bacc.Bacc(
        "TRN2",
        target_bir_lowering=False,
        debug=False,
        num_devices=NCORES,
    )

    xr_d = nc.dram_tensor("xr", [128, BC], f32r, kind="ExternalInput")
    w1s_d = nc.dram_tensor("w1s", [128, NROUND * 128], f32r, kind="ExternalInput")
    w2s_d = nc.dram_tensor("w2s", [128, NCHUNK * 32], bf16, kind="ExternalInput")
    cmat_d = nc.dram_tensor("cmat", [128, 16], f32r, kind="ExternalInput")
    tiny_d = nc.dram_tensor("tiny", [32, 16], f32r, kind="ExternalInput")
    y_d = nc.dram_tensor("y", [16, BC], f32, kind="ExternalOutput")

    with tile.TileContext(nc) as tc:
        with (
            tc.tile_pool(name="consts", bufs=1) as consts,
            tc.tile_pool(name="xp", bufs=2) as xp,
            tc.tile_pool(name="hsb", bufs=4) as hsb,
            tc.tile_pool(name="stkp", bufs=2) as stkp,
            tc.tile_pool(name="yp", bufs=2) as yp,
            tc.tile_pool(name="hps", bufs=3, space="PSUM") as hps,
            tc.tile_pool(name="accp", bufs=1, space="PSUM") as accp,
            tc.tile_pool(name="outp", bufs=1, space="PSUM") as outp,
        ):
            w1s = consts.tile([128, NROUND * 128], f32r, name="w1s_sb")
            w2s = consts.tile([128, NCHUNK * 32], bf16, name="w2s_sb")
            cmat = consts.tile([128, 16], f32r, name="cmat_sb")
            tiny = consts.tile([32, 16], f32r, name="tiny_sb")
            nc.sync.dma_start(w1s[:], w1s_d[:])
            nc.sync.dma_start(w2s[:], w2s_d[:])
            nc.sync.dma_start(cmat[:], cmat_d[:])
            nc.sync.dma_start(tiny[:], tiny_d[:])

            for rep in range(reps):
              for bt in range(NT):
                bsl = slice(bt * TB, (bt + 1) * TB)
                xt = xp.tile([128, TB], f32r, name="xt", tag="xt")
                nc.sync.dma_start(xt[:], xr_d[:, bsl])

                acc = accp.tile([128, TB], f32, name="acc", tag="acc")

                for r in range(NROUND):
                    # L1: 4 row-tiled matmuls -> 2 psum pair-tiles
                    # (2 chunks each, 2 banks each)
                    pair = []
                    for p in range(2):
                        hp = hps.tile([128, 2 * TB], f32, name=f"hp{p}", tag="hp")
                        pair.append(hp)
                    for i in range(4):
                        hp = pair[i // 2]
                        osl = slice((i % 2) * TB, (i % 2 + 1) * TB)
                        nc.tensor.matmul(
                            hp[:, osl],
                            w1s[32 * i : 32 * i + 32, 128 * r : 128 * r + 128],
                            xt[32 * i : 32 * i + 32, :],
                            start=True,
                            stop=True,
                            tile_position=(32 * i, 0),
                        )
                    # relu PSUM->SBUF: one [128,1024] op per engine
                    hs = []
                    for p in range(2):
                        ht = hsb.tile([128, 2 * TB], bf16, name=f"hs{p}", tag=f"hs{p}")
                        use_act = (p == 0) ^ (r % 2 == 1)
                        if use_act:
                            nc.scalar.activation(ht[:], pair[p][:], Relu)
                        else:
                            nc.vector.tensor_scalar_max(ht[:], pair[p][:], 0.0)
                        hs.append(ht)
                    # L2: 4 col-tiled matmuls accumulating into acc slices
                    for j in range(4):
                        c = 4 * r + j
                        ht = hs[j // 2]
                        rsl = slice((j % 2) * TB, (j % 2 + 1) * TB)
                        nc.tensor.matmul(
                            acc[32 * j : 32 * j + 32, :],
                            w2s[:, 32 * c : 32 * c + 32],
                            ht[:, rsl],
                            start=(r == 0),
                            stop=(r == NROUND - 1),
                            tile_position=(0, 32 * j),
                            skip_group_check=True,
                        )

                # collapse 4 slices + tiny folded path -> [16, TB]
                stk = stkp.tile([128, TB], f32r, name="stk", tag="stk")
                nc.vector.tensor_copy(stk[:], acc[:])
                ops = outp.tile([16, TB], f32, name="ops", tag="ops")
                nc.tensor.matmul(
                    ops[:],
                    cmat[:],
                    stk[:],
                    start=True,
                    stop=False,
                    skip_group_check=True,
                )
                nc.tensor.matmul(
                    ops[:],
                    tiny[:],
                    xt[0:32, :],
                    start=False,
                    stop=True,
                    skip_group_check=True,
                )
                yt = yp.tile([16, TB], f32, name="yt", tag="yt")
                nc.vector.tensor_copy(yt[:], ops[:])
                nc.sync.dma_start(y_d[:, bsl], yt[:])

    nc.compile()
    _cache[key] = nc
    return nc


def _prep_inputs(x, W1, b1, W2, b2):
    """Build per-core in_maps (host-side shard + weight folding)."""
    x = np.asarray(x, dtype=np.float32)
    W1 = np.asarray(W1, dtype=np.float32)
    b1 = np.asarray(b1, dtype=np.float32)
    W2 = np.asarray(W2, dtype=np.float32)
    b2 = np.asarray(b2, dtype=np.float32)

    W1f = W1.reshape(NH, I)              # [8192, 16]
    b1f = b1.reshape(NH)                 # [8192]

    # w1s: per round r, row-block i holds chunk c=4r+i as lhsT [32, 128]:
    # rows 0:16 = W1f[chunk].T, row 16 = b1f[chunk], rows 17:32 = 0
    w1s = np.zeros((128, NROUND * 128), dtype=np.float32)
    for c in range(NCHUNK):
        r, i = divmod(c, 4)
        blk = slice(128 * c, 128 * c + 128)
        w1s[32 * i : 32 * i + 16, 128 * r : 128 * r + 128] = W1f[blk].T
        w1s[32 * i + 16, 128 * r : 128 * r + 128] = b1f[blk]

    # w2s: per chunk c (head r=c//4, quarter j=c%4): [128, 32] block, only
    # column r nonzero = 0.8 * W2[r, 128j : 128j+128]
    import ml_dtypes

    w2s = np.zeros((128, NCHUNK * 32), dtype=np.float32)
    for c in range(NCHUNK):
        r, j = divmod(c, 4)
        w2s[:, 32 * c + r] = 0.8 * W2[r, 128 * j : 128 * j + 128]
    w2s = w2s.astype(ml_dtypes.bfloat16)

    # collapse: sum the 4 col-tile slices
    cmat = np.zeros((128, 16), dtype=np.float32)
    for a in range(4):
        for h in range(16):
            cmat[32 * a + h, h] = 1.0

    # tiny folded linear path: 0.2 * W2^T (W1 x + b1) + b2
    tiny = np.zeros((32, 16), dtype=np.float32)
    for o in range(O):
        tiny[0:16, o] = 0.2 * (W2[o] @ W1[o])
        tiny[16, o] = 0.2 * float(W2[o] @ b1[o]) + float(b2[o])

    in_maps = []
    for core in range(NCORES):
        xc = x[core * BC : (core + 1) * BC]          # [4096, 16]
        xa = np.zeros((32, BC), dtype=np.float32)
        xa[0:16] = xc.T
        xa[16] = 1.0
        xr = np.tile(xa, (4, 1))                     # [128, 4096]
        in_maps.append(
            {
                "xr": np.ascontiguousarray(xr),
                "w1s": w1s,
                "w2s": w2s,
                "cmat": cmat,
                "tiny": tiny,
            }
        )
    return in_maps


last_results = None


def kernel(x, W1, b1, W2, b2):
    global last_results
    from concourse.bass_utils import run_bass_kernel_spmd

    nc = _build()
    in_maps = _prep_inputs(x, W1, b1, W2, b2)
    res = run_bass_kernel_spmd(nc, in_maps, core_ids=list(range(NCORES)))
    last_results = res
    out = np.empty((B, O), dtype=np.float32)
    for core in range(NCORES):
        out[core * BC : (core + 1) * BC] = res.results[core]["y"].T
    return out

